# revision 7
# baseline (speedup 1.0000x reference)
"""Trainium2 Bass kernel for nn_FFDGenerator: cubic B-spline free-form deformation.

Computes flow[b,c,x,y,z] = sum_{i,j,k} Wx[x,i]*Wy[y,j]*Wz[z,k]*mesh[b,c,i,j,k]
where Wx/Wy/Wz are dense per-axis cubic B-spline weight matrices (4 nonzeros
per row, spacing 8), mesh is (4,3,23,27,23), flow is (4,3,160,192,160).

Sharding: output x-axis split into 8 chunks of 20, one per NeuronCore.
Control-point mesh is replicated (bc-triples spread over partition bands).

Per-core pipeline (all contractions on the tensor engine):
  MM1: contract i -> A[xl, (bc,j,k)]       col-tiled over 4 partition bands
  T:   DVE 32x32 block transpose -> A_t[k, (bc,xl,j)] per band
  MM2: contract k -> C[(xl4,j32), (g,z)]   row-banded, data-stationary
  MM3: contract j -> out[(xl4,y32), z]     block-diag Wy weights, M=128
  Epilogue: PSUM -> SBUF fp16 copy with 1/27 scale, DMA to DRAM.

Numerics: single-pass fp16 matmuls (tolerance is 2e-2; this lands ~5e-4).
Weights are exact in fp16 via the 3x trick: 3*W has entries n/1024 with
|n| <= 2048; the 27x scale is removed in the epilogue copy. The output
crosses the wire as fp16 and is upcast to fp32 on the host, halving the
dominant HBM store traffic.
"""

import numpy as np

import concourse.bass as bass
import concourse.mybir as mybir
from concourse.tile import TileContext
from concourse.bass_utils import run_bass_kernel_spmd

F16 = mybir.dt.float16
F32 = mybir.dt.float32

NCORES = 8
B, C = 4, 3
BC = B * C                    # 12 bc slices
X, Y, Z = 160, 192, 160
XL = X // NCORES              # 20 x per core
CX, CY, CZ = 23, 27, 23       # control points per axis
J32 = 32                      # padded j
K32 = 32                      # padded k
NB = 4                        # partition bands
BAND_BC = 3                   # bc per band
FREE1 = BAND_BC * J32 * K32   # 3072: per-band free size of meshT/A
NGRP = BC * XL // 4           # 60 groups of 4 (bc,x) slices
NYC = Y // 32                 # 6 y-chunks
GG = 3                        # groups per MM3 supertile
NGP = NGRP // GG              # 20 supertiles
CHUNK = 512                   # MM1 free chunk (one PSUM bank of fp32)
NCH = FREE1 // CHUNK          # 6 chunks

_cache = {}


def _axis_weights3(n, sp, ncp):
    """Dense [n, ncp] matrix of 3x the cubic B-spline weights.

    With integer coordinates and spacing 8, 3*w = m/1024 with |m| <= 2048,
    exactly representable in fp16.
    """
    v = np.arange(n, dtype=np.float64) / sp
    f = np.floor(v)
    d = v - f
    w = np.stack(
        [
            (1 - d) ** 3 / 6,
            d**3 / 2 - d**2 + 2.0 / 3,
            -(d**3) / 2 + d**2 / 2 + d / 2 + 1.0 / 6,
            d**3 / 6,
        ],
        -1,
    )
    W3 = np.zeros((n, ncp))
    idx = f.astype(int)
    for a in range(4):
        W3[np.arange(n), idx + a] = np.round(3 * w[:, a] * 1024) / 1024
    assert np.all(np.float64(np.float16(W3)) == W3)
    return W3


def _host_weights():
    if "w" in _cache:
        return _cache["w"]
    W3x = _axis_weights3(X, 8, CX)
    W3y = _axis_weights3(Y, 8, CY)
    W3z = _axis_weights3(Z, 8, CZ)

    # wx3[core]: [128, 32] fp16, wxT chunk replicated on 4 partition bands
    wx3 = np.zeros((NCORES, 128, 32), np.float16)
    for core in range(NCORES):
        blk = W3x[core * XL : (core + 1) * XL, :].T.astype(np.float16)  # [23, 20]
        for q in range(NB):
            wx3[core, 32 * q : 32 * q + CX, :XL] = blk

    # wz3: [128, Z] fp16, wzT replicated on 4 partition bands
    wz3 = np.zeros((128, Z), np.float16)
    for q in range(NB):
        wz3[32 * q : 32 * q + CZ, :] = W3z.T.astype(np.float16)

    # wyb: [128, NYC*128] fp16 block-diag. Variant c covers the strided y-set
    # y = 6*yi + c (yi = 0..31): wyb[32b+j, 128c + 32b+yi] = W3y[6yi+c, j].
    # The stride-6 y interleave makes each slice's (yi, c, z) staging layout
    # land contiguously in flow[y, z], so one DMA stores a whole supertile.
    wyb = np.zeros((128, NYC * 128), np.float16)
    for c in range(NYC):
        for b in range(4):
            for j in range(CY):
                wyb[32 * b + j, 128 * c + 32 * b : 128 * c + 32 * b + 32] = W3y[
                    c : c + 6 * 32 : 6, j
                ].astype(np.float16)

    _cache["w"] = (wx3, wz3, wyb)
    return _cache["w"]


def _prep_mesh(mesh):
    """mesh [4,3,23,27,23] f32 -> meshT [128, FREE1] fp16.

    Partition 32q+i holds mesh[bc=3q+bcq, i, j, k] at free index
    bcq*J32*K32 + j*K32 + k (j, k zero-padded to 32).
    """
    m = np.asarray(mesh, np.float32).reshape(BC, CX, CY, CZ)
    mt = np.zeros((128, BAND_BC, J32, K32), np.float32)
    for q in range(NB):
        for bcq in range(BAND_BC):
            bc = 3 * q + bcq
            mt[32 * q : 32 * q + CX, bcq, :CY, :CZ] = m[bc]
    return mt.astype(np.float16).reshape(128, FREE1)


def _build_program():
    if "nc" in _cache:
        return _cache["nc"]
    nc = bass.Bass()
    mesh16 = nc.declare_dram_parameter("mesh16", [128, FREE1], F16, isOutput=False)
    wx3 = nc.declare_dram_parameter("wx3", [128, 32], F16, isOutput=False)
    wz3 = nc.declare_dram_parameter("wz3", [128, Z], F16, isOutput=False)
    wyb = nc.declare_dram_parameter("wyb", [128, NYC * 128], F16, isOutput=False)
    flow = nc.declare_dram_parameter("flow", [BC * XL, Y, Z], F16, isOutput=True)

    # Store view: slice s = 12*gp + 4*gg + b, y = 6*yi + c. With staging laid
    # out [(b,yi) partitions, (gg,c,z) free], a whole supertile is one DMA
    # whose SBUF side is fully contiguous per partition (3x1920B dst runs).
    flowV = flow[:, :, :].rearrange(
        "(gp gg b) (yi c) z -> gp (b yi) gg c z", gg=GG, b=4, c=NYC
    )

    with TileContext(nc) as tc:
        with (
            tc.tile_pool(name="const", bufs=1) as cpool,
            tc.tile_pool(name="abuf", bufs=1) as apool,
            tc.tile_pool(name="cbuf", bufs=4) as cbpool,
            tc.tile_pool(name="stage", bufs=6) as spool,
            tc.tile_pool(name="ps1", bufs=2, space="PSUM") as ps1pool,
            tc.tile_pool(name="ps2", bufs=2, space="PSUM") as ps2pool,
            tc.tile_pool(name="ps3", bufs=2, space="PSUM") as ps3pool,
        ):
            # Per-chunk mesh tiles so MM1 chunk ch starts as soon as its own
            # slice of the mesh has landed (Tile deps are whole-tile).
            wx = cpool.tile([128, 32], F16, tag="wx")
            nc.sync.dma_start(out=wx[:, :], in_=wx3[:, :])
            m2c = []
            for ch in range(NCH):
                s = slice(ch * CHUNK, (ch + 1) * CHUNK)
                t2 = cpool.tile([128, CHUNK], F16, name=f"m2{ch}", tag=f"m2{ch}")
                eng = nc.sync if ch % 2 == 0 else nc.gpsimd
                eng.dma_start(out=t2[:, :], in_=mesh16[:, s])
                m2c.append(t2)
            wz = cpool.tile([128, Z], F16, tag="wz")
            wy = cpool.tile([128, NYC * 128], F16, tag="wy")
            nc.sync.dma_start(out=wz[:, :], in_=wz3[:, :])
            nc.gpsimd.dma_start(out=wy[:, :], in_=wyb[:, :])

            # ---- MM1 (contract i) + 32x32 block transpose ----
            # Separate A tiles per bc-triple (bcq) so MM2 groups that consume
            # one triple can start while later chunks are still in MM1.
            PB = J32 * K32  # 1024: per-bcq free size
            at = [apool.tile([128, PB], F32, name=f"at{b}", tag=f"at{b}")
                  for b in range(BAND_BC)]
            # fp16 copy, stored permuted (j,x)->(x,j) so each MM2 lhsT is one
            # contiguous 128-wide run (walrus: 1 free dim).
            ah = [apool.tile([128, PB], F16, name=f"ah{b}", tag=f"ah{b}")
                  for b in range(BAND_BC)]
            for ch in range(NCH):
                p1 = ps1pool.tile([128, CHUNK], F32, tag="p1")
                for q in range(NB):
                    band = slice(32 * q, 32 * q + CX)
                    nc.tensor.matmul(
                        p1[32 * q : 32 * q + 32, :],
                        lhsT=wx[band, :],
                        rhs=m2c[ch][band, :],
                        start=True,
                        stop=True,
                        tile_position=(32 * q, 32 * q),
                    )
                bq, half = ch // 2, (ch % 2) * CHUNK
                nc.vector.transpose(
                    out=at[bq][:, half : half + CHUNK], in_=p1[:, :]
                )
                if ch % 2 == 1:
                    b = bq
                    atP = at[b][:, :].rearrange(
                        "p (j x) -> p x j", j=J32, x=K32
                    )
                    ahV = ah[b][:, :].rearrange(
                        "p (x j) -> p x j", x=K32, j=J32
                    )
                    # SBUF->SBUF, so the PSUM-incapable gpsimd engine can
                    # own it, keeping scalar/vector free for PSUM drains.
                    nc.gpsimd.tensor_scalar_mul(ahV, atP, 1.0)

            # ---- MM2 (contract k) + C fp16 copy + MM3 (contract j) + store --
            # Fully interleaved per gp (= one supertile of 3 slice quads):
            # MM2 produces C for this gp, MM3 consumes it immediately; the
            # fp16 staging tile is stored with one 737KB DMA per supertile.
            inv27 = float(1.0 / 27.0)

            def emit_mm2(gp):
                """Produce C (fp16) for supertile gp."""
                p2 = ps2pool.tile([128, GG * Z], F32, tag="p2", name="p2")
                for sub in range(GG):
                    g = gp * GG + sub
                    bc = g // 5
                    q, bq, xg = bc // 3, bc % 3, g % 5
                    lo = 128 * xg
                    nc.tensor.matmul(
                        p2[:, sub * Z : (sub + 1) * Z],
                        lhsT=ah[bq][32 * q : 32 * q + CZ, lo : lo + 128],
                        rhs=wz[32 * q : 32 * q + CZ, :],
                        start=True,
                        stop=True,
                        tile_position=(32 * q, 0),
                    )
                chl = cbpool.tile([128, GG * Z], F16, name="chl", tag="chl")
                # PSUM drain: only scalar/vector may read PSUM. 2:1 split
                # matches their (slower:faster) element rates.
                if gp % 3 == 2:
                    nc.vector.tensor_scalar_mul(chl[:, :], p2[:, :], 1.0)
                else:
                    nc.scalar.copy(out=chl[:, :], in_=p2[:, :])
                return chl

            def emit_mm3(gp, chl):
                """Contract j for supertile gp, scale to fp16, and store."""
                stg = spool.tile([128, GG * NYC * Z], F16, tag="stg", name="stg")
                for cp in range(NYC // 2):  # pairs of y-variants
                    # two bank-aligned [128,480] halves in a 2-bank tile
                    p3 = ps3pool.tile([128, 1024], F32, tag="p3", name="p3")
                    for cc in range(2):
                        c = 2 * cp + cc
                        nc.tensor.matmul(
                            p3[:, cc * 512 : cc * 512 + GG * Z],
                            lhsT=wy[:, 128 * c : 128 * (c + 1)],
                            rhs=chl[:, :],
                            start=True,
                            stop=True,
                        )
                    # scale-copy into the (gg, c, z) staging layout; one copy
                    # per cc half: src [p, gg, z], dst strided over gg.
                    for cc in range(2):
                        c = 2 * cp + cc
                        srcb = p3[:, cc * 512 : cc * 512 + GG * Z]
                        src = bass.AP(
                            srcb.tensor, srcb.offset,
                            [srcb.ap[0], [Z, GG], [1, Z]],
                        )
                        dstb = stg[:, c * Z : c * Z + Z]
                        dst = bass.AP(
                            dstb.tensor, dstb.offset,
                            [dstb.ap[0], [NYC * Z, GG], [1, Z]],
                        )
                        if cp == 0:
                            nc.scalar.mul(dst, src, inv27)
                        else:
                            nc.vector.tensor_scalar_mul(dst, src, inv27)
                stgV = stg[:, :].rearrange(
                    "p (gg c z) -> p gg c z", gg=GG, c=NYC
                )
                eng = nc.sync if gp % 2 == 0 else nc.gpsimd
                eng.dma_start(out=flowV[gp], in_=stgV)

            # Software pipeline (depth 2): MM3 for gp-2 is emitted after MM2
            # for gp, so C production runs well ahead of its consumption.
            pend = []
            for gp in range(NGP):
                pend.append((gp, emit_mm2(gp)))
                if len(pend) > 2:
                    emit_mm3(*pend.pop(0))
            for item in pend:
                emit_mm3(*item)

    # Walrus allows at most one sync-wait per matmul; split extras into
    # EventSemaphore instructions (same pass Bacc.compile runs).
    import bass_rust as _bass_rust

    _bass_rust.move_matmul_waits_to_ldweights(nc.m)
    _bass_rust.generate_event_semaphores(nc)

    _cache["nc"] = nc
    return nc


def _in_maps(mesh):
    wx3, wz3, wyb = _host_weights()
    mh = _prep_mesh(mesh)
    return [
        {"mesh16": mh, "wx3": wx3[core], "wz3": wz3, "wyb": wyb}
        for core in range(NCORES)
    ]


def kernel(mesh: np.ndarray) -> np.ndarray:
    nc = _build_program()
    in_maps = _in_maps(mesh)
    last_err = None
    for attempt in range(3):
        try:
            res = run_bass_kernel_spmd(nc, in_maps, list(range(NCORES))).results
            break
        except Exception as e:  # transient device wedge: retry
            last_err = e
    else:
        raise last_err
    full = np.empty((BC, X, Y, Z), np.float32)
    for core in range(NCORES):
        full[:, core * XL : (core + 1) * XL] = res[core]["flow"].reshape(
            BC, XL, Y, Z
        )
    return full.reshape(B, C, X, Y, Z)


# revision 12
# speedup vs baseline: 1.6715x; 1.6715x over previous
"""Trainium2 Bass kernel for nn_FFDGenerator: cubic B-spline free-form deformation.

Computes flow[b,c,x,y,z] = sum_{i,j,k} Wx[x,i]*Wy[y,j]*Wz[z,k]*mesh[b,c,i,j,k]
where Wx/Wy/Wz are dense per-axis cubic B-spline weight matrices (4 nonzeros
per row, spacing 8), mesh is (4,3,23,27,23), flow is (4,3,160,192,160).

Sharding: output x-axis split into 8 chunks of 20, one per NeuronCore.
Control-point mesh is replicated (bc-triples spread over partition bands).

Per-core pipeline (all contractions on the tensor engine):
  MM1: contract i -> A[xl, (bc,j,k)]       col-tiled over 4 partition bands
  T:   DVE 32x32 block transpose -> A_t[k, (bc,xl,j)] per band
  MM2: contract k -> C[(xl4,j32), (g,z)]   row-banded, data-stationary
  MM3: contract j -> out[(xl4,y32), z]     block-diag Wy weights, M=128
  Epilogue: PSUM -> SBUF fp16 copy with 1/27 scale, DMA to DRAM.

Numerics: single-pass fp16 matmuls (tolerance is 2e-2; this lands ~5e-4).
Weights are exact in fp16 via the 3x trick: 3*W has entries n/1024 with
|n| <= 2048; the 27x scale is removed in the epilogue copy. The output
crosses the wire as fp16 and is upcast to fp32 on the host, halving the
dominant HBM store traffic.
"""

import numpy as np

import concourse.bass as bass
import concourse.mybir as mybir
from concourse.tile import TileContext
from concourse.bass_utils import run_bass_kernel_spmd

F16 = mybir.dt.float16
F32 = mybir.dt.float32

NCORES = 8
B, C = 4, 3
BC = B * C                    # 12 bc slices
X, Y, Z = 160, 192, 160
XL = X // NCORES              # 20 x per core
CX, CY, CZ = 23, 27, 23       # control points per axis
J32 = 32                      # padded j
K32 = 32                      # padded k
NB = 4                        # partition bands
BAND_BC = 3                   # bc per band
FREE1 = BAND_BC * J32 * K32   # 3072: per-band free size of meshT/A
NGRP = BC * XL // 4           # 60 groups of 4 (bc,x) slices
NYC = Y // 32                 # 6 y-chunks
GG = 3                        # groups per MM3 supertile
NGP = NGRP // GG              # 20 supertiles
CHUNK = 512                   # MM1 free chunk (one PSUM bank of fp32)
NCH = FREE1 // CHUNK          # 6 chunks

_cache = {}


def _axis_weights3(n, sp, ncp):
    """Dense [n, ncp] matrix of 3x the cubic B-spline weights.

    With integer coordinates and spacing 8, 3*w = m/1024 with |m| <= 2048,
    exactly representable in fp16.
    """
    v = np.arange(n, dtype=np.float64) / sp
    f = np.floor(v)
    d = v - f
    w = np.stack(
        [
            (1 - d) ** 3 / 6,
            d**3 / 2 - d**2 + 2.0 / 3,
            -(d**3) / 2 + d**2 / 2 + d / 2 + 1.0 / 6,
            d**3 / 6,
        ],
        -1,
    )
    W3 = np.zeros((n, ncp))
    idx = f.astype(int)
    for a in range(4):
        W3[np.arange(n), idx + a] = np.round(3 * w[:, a] * 1024) / 1024
    assert np.all(np.float64(np.float16(W3)) == W3)
    return W3


def _host_weights():
    if "w" in _cache:
        return _cache["w"]
    W3x = _axis_weights3(X, 8, CX)
    W3y = _axis_weights3(Y, 8, CY)
    W3z = _axis_weights3(Z, 8, CZ)

    # wx3[core]: [128, 32] fp16, wxT chunk replicated on 4 partition bands
    wx3 = np.zeros((NCORES, 128, 32), np.float16)
    for core in range(NCORES):
        blk = W3x[core * XL : (core + 1) * XL, :].T.astype(np.float16)  # [23, 20]
        for q in range(NB):
            wx3[core, 32 * q : 32 * q + CX, :XL] = blk

    # wz3: [128, Z] fp16, wzT replicated on 4 partition bands
    wz3 = np.zeros((128, Z), np.float16)
    for q in range(NB):
        wz3[32 * q : 32 * q + CZ, :] = W3z.T.astype(np.float16)

    # wyb: [128, NYC*128] fp16 block-diag. Variant c covers the strided y-set
    # y = 6*yi + c (yi = 0..31): wyb[32b+j, 128c + 32b+yi] = W3y[6yi+c, j].
    # The stride-6 y interleave makes each slice's (yi, c, z) staging layout
    # land contiguously in flow[y, z], so one DMA stores a whole supertile.
    wyb = np.zeros((128, NYC * 128), np.float16)
    for c in range(NYC):
        for b in range(4):
            for j in range(CY):
                wyb[32 * b + j, 128 * c + 32 * b : 128 * c + 32 * b + 32] = W3y[
                    c : c + 6 * 32 : 6, j
                ].astype(np.float16)

    _cache["w"] = (wx3, wz3, wyb)
    return _cache["w"]


def _prep_mesh(mesh):
    """mesh [4,3,23,27,23] f32 -> meshT [128, FREE1] fp16.

    Partition 32q+i holds mesh[bc=3q+bcq, i, j, k] at free index
    bcq*J32*K32 + j*K32 + k (j, k zero-padded to 32).
    """
    m = np.asarray(mesh, np.float32).reshape(BC, CX, CY, CZ)
    mt = np.zeros((128, BAND_BC, J32, K32), np.float32)
    for q in range(NB):
        for bcq in range(BAND_BC):
            bc = 3 * q + bcq
            mt[32 * q : 32 * q + CX, bcq, :CY, :CZ] = m[bc]
    return mt.astype(np.float16).reshape(128, FREE1)


def _build_program():
    if "nc" in _cache:
        return _cache["nc"]
    nc = bass.Bass()
    mesh16 = nc.declare_dram_parameter("mesh16", [128, FREE1], F16, isOutput=False)
    wx3 = nc.declare_dram_parameter("wx3", [128, 32], F16, isOutput=False)
    wz3 = nc.declare_dram_parameter("wz3", [128, Z], F16, isOutput=False)
    wyb = nc.declare_dram_parameter("wyb", [128, NYC * 128], F16, isOutput=False)
    flow = nc.declare_dram_parameter("flow", [BC * XL, Y, Z], F16, isOutput=True)

    # Store view: slice s = 12*gp + 4*gg + b, y = 6*yi + c. With staging laid
    # out [(b,yi) partitions, (gg,c,z) free], a whole supertile is one DMA
    # whose SBUF side is fully contiguous per partition (3x1920B dst runs).
    flowV = flow[:, :, :].rearrange(
        "(gp gg b) (yi c) z -> gp (b yi) gg c z", gg=GG, b=4, c=NYC
    )

    with TileContext(nc) as tc:
        with (
            tc.tile_pool(name="const", bufs=1) as cpool,
            tc.tile_pool(name="abuf", bufs=1) as apool,
            tc.tile_pool(name="cbuf", bufs=4) as cbpool,
            tc.tile_pool(name="stage", bufs=6) as spool,
            tc.tile_pool(name="ps12", bufs=2, space="PSUM") as ps12pool,
            tc.tile_pool(name="ps3", bufs=2, space="PSUM") as ps3pool,
        ):
            # Per-chunk mesh tiles so MM1 chunk ch starts as soon as its own
            # slice of the mesh has landed (Tile deps are whole-tile).
            wx = cpool.tile([128, 32], F16, tag="wx")
            nc.sync.dma_start(out=wx[:, :], in_=wx3[:, :])
            m2c = []
            for ch in range(NCH):
                s = slice(ch * CHUNK, (ch + 1) * CHUNK)
                t2 = cpool.tile([128, CHUNK], F16, name=f"m2{ch}", tag=f"m2{ch}")
                eng = nc.sync if ch % 2 == 0 else nc.gpsimd
                eng.dma_start(out=t2[:, :], in_=mesh16[:, s])
                m2c.append(t2)
            wz = cpool.tile([128, Z], F16, tag="wz")
            wy = cpool.tile([128, NYC * 128], F16, tag="wy")
            nc.sync.dma_start(out=wz[:, :], in_=wz3[:, :])
            nc.gpsimd.dma_start(out=wy[:, :], in_=wyb[:, :])

            # ---- MM1 (contract i) + 32x32 block transpose ----
            # Separate A tiles per bc-triple (bcq) so MM2 groups that consume
            # one triple can start while later chunks are still in MM1.
            PB = J32 * K32  # 1024: per-bcq free size
            at = [apool.tile([128, PB], F32, name=f"at{b}", tag=f"at{b}")
                  for b in range(BAND_BC)]
            # fp16 copy, stored permuted (j,x)->(x,j) so each MM2 lhsT is one
            # contiguous 128-wide run (walrus: 1 free dim).
            ah = [apool.tile([128, PB], F16, name=f"ah{b}", tag=f"ah{b}")
                  for b in range(BAND_BC)]
            for ch in range(NCH):
                p1 = ps12pool.tile([128, CHUNK], F32, tag="p12", name="p1")
                for q in range(NB):
                    band = slice(32 * q, 32 * q + CX)
                    nc.tensor.matmul(
                        p1[32 * q : 32 * q + 32, :],
                        lhsT=wx[band, :],
                        rhs=m2c[ch][band, :],
                        start=True,
                        stop=True,
                        tile_position=(32 * q, 32 * q),
                    )
                bq, half = ch // 2, (ch % 2) * CHUNK
                nc.vector.transpose(
                    out=at[bq][:, half : half + CHUNK], in_=p1[:, :]
                )
                if ch % 2 == 1:
                    b = bq
                    atP = at[b][:, :].rearrange(
                        "p (j x) -> p x j", j=J32, x=K32
                    )
                    ahV = ah[b][:, :].rearrange(
                        "p (x j) -> p x j", x=K32, j=J32
                    )
                    nc.scalar.copy(out=ahV, in_=atP)

            # ---- MM2 (contract k) + C fp16 copy + MM3 (contract j) + store --
            # Fully interleaved per gp (= one supertile of 3 slice quads):
            # MM2 produces C for this gp, MM3 consumes it immediately; the
            # fp16 staging tile is stored with one 737KB DMA per supertile.
            inv27 = float(1.0 / 27.0)

            def emit_mm2(gp):
                """Produce C (fp16) for supertile gp."""
                p2 = ps12pool.tile([128, CHUNK], F32, tag="p12", name="p2")[:, : GG * Z]
                for sub in range(GG):
                    g = gp * GG + sub
                    bc = g // 5
                    q, bq, xg = bc // 3, bc % 3, g % 5
                    lo = 128 * xg
                    nc.tensor.matmul(
                        p2[:, sub * Z : (sub + 1) * Z],
                        lhsT=ah[bq][32 * q : 32 * q + CZ, lo : lo + 128],
                        rhs=wz[32 * q : 32 * q + CZ, :],
                        start=True,
                        stop=True,
                        tile_position=(32 * q, 0),
                    )
                chl = cbpool.tile([128, GG * Z], F16, name="chl", tag="chl")
                # PSUM drain: only scalar/vector may read PSUM.
                if gp % 3 == 2:
                    nc.vector.tensor_scalar_mul(chl[:, :], p2[:, :], 1.0)
                else:
                    nc.scalar.copy(out=chl[:, :], in_=p2[:, :])
                return chl

            def emit_mm3(gp, chl):
                """Contract j for supertile gp, scale to fp16, and store."""
                stg = spool.tile([128, GG * NYC * Z], F16, tag="stg", name="stg")
                for h in range(2):  # c-triples {0,1,2} and {3,4,5}
                    # three bank-aligned [128,480] thirds in a 3-bank tile
                    p3 = ps3pool.tile([128, 1536], F32, tag="p3", name="p3")
                    for cc in range(3):
                        c = 3 * h + cc
                        nc.tensor.matmul(
                            p3[:, cc * 512 : cc * 512 + GG * Z],
                            lhsT=wy[:, 128 * c : 128 * (c + 1)],
                            rhs=chl[:, :],
                            start=True,
                            stop=True,
                        )
                    # One fused scale-copy drains the whole c-triple into the
                    # (gg, c, z) staging layout. Iteration (gg, cc, z): dst
                    # runs are 480 contiguous fp16 per gg; src hops PSUM
                    # banks on cc.
                    srcb = p3[:, :]
                    src = bass.AP(
                        srcb.tensor, srcb.offset,
                        [srcb.ap[0], [Z, GG], [512, 3], [1, Z]],
                    )
                    dstb = stg[:, 3 * h * Z : 3 * h * Z + Z]
                    dst = bass.AP(
                        dstb.tensor, dstb.offset,
                        [dstb.ap[0], [NYC * Z, GG], [Z, 3], [1, Z]],
                    )
                    if (2 * gp + h) % 5 < 3:
                        nc.vector.tensor_scalar_mul(dst, src, inv27)
                    else:
                        nc.scalar.mul(dst, src, inv27)
                stgV = stg[:, :].rearrange(
                    "p (gg c z) -> p gg c z", gg=GG, c=NYC
                )
                eng = nc.sync if gp % 2 == 0 else nc.gpsimd
                eng.dma_start(out=flowV[gp], in_=stgV)

            # Software pipeline (depth 2): MM3 for gp-2 is emitted after MM2
            # for gp, so C production runs well ahead of its consumption.
            pend = []
            for gp in range(NGP):
                pend.append((gp, emit_mm2(gp)))
                if len(pend) > 2:
                    emit_mm3(*pend.pop(0))
            for item in pend:
                emit_mm3(*item)

    # Walrus allows at most one sync-wait per matmul; split extras into
    # EventSemaphore instructions (same pass Bacc.compile runs).
    import bass_rust as _bass_rust

    _bass_rust.move_matmul_waits_to_ldweights(nc.m)
    _bass_rust.generate_event_semaphores(nc)

    _cache["nc"] = nc
    return nc


def _in_maps(mesh):
    wx3, wz3, wyb = _host_weights()
    mh = _prep_mesh(mesh)
    return [
        {"mesh16": mh, "wx3": wx3[core], "wz3": wz3, "wyb": wyb}
        for core in range(NCORES)
    ]


def kernel(mesh: np.ndarray) -> np.ndarray:
    nc = _build_program()
    in_maps = _in_maps(mesh)
    last_err = None
    for attempt in range(3):
        try:
            res = run_bass_kernel_spmd(nc, in_maps, list(range(NCORES))).results
            break
        except Exception as e:  # transient device wedge: retry
            last_err = e
    else:
        raise last_err
    full = np.empty((BC, X, Y, Z), np.float32)
    for core in range(NCORES):
        full[:, core * XL : (core + 1) * XL] = res[core]["flow"].reshape(
            BC, XL, Y, Z
        )
    return full.reshape(B, C, X, Y, Z)


# revision 16
# speedup vs baseline: 1.7059x; 1.0206x over previous
"""Trainium2 Bass kernel for nn_FFDGenerator: cubic B-spline free-form deformation.

Computes flow[b,c,x,y,z] = sum_{i,j,k} Wx[x,i]*Wy[y,j]*Wz[z,k]*mesh[b,c,i,j,k]
where Wx/Wy/Wz are dense per-axis cubic B-spline weight matrices (4 nonzeros
per row, spacing 8), mesh is (4,3,23,27,23), flow is (4,3,160,192,160).

Sharding: output x-axis split into 8 chunks of 20, one per NeuronCore.
Control-point mesh is replicated (bc-triples spread over partition bands).

Per-core pipeline (all contractions on the tensor engine):
  MM1: contract i -> A[xl, (bc,j,k)]       col-tiled over 4 partition bands
  T:   DVE 32x32 block transpose -> A_t[k, (bc,xl,j)] per band
  MM2: contract k -> C[(xl4,j32), (g,z)]   row-banded, data-stationary
  MM3: contract j -> out[(xl4,y32), z]     block-diag Wy weights, M=128
  Epilogue: PSUM -> SBUF fp16 copy with 1/27 scale, DMA to DRAM.

Numerics: single-pass fp16 matmuls (tolerance is 2e-2; this lands ~5e-4).
Weights are exact in fp16 via the 3x trick: 3*W has entries n/1024 with
|n| <= 2048; the 27x scale is removed in the epilogue copy. The output
crosses the wire as fp16 and is upcast to fp32 on the host, halving the
dominant HBM store traffic.
"""

import numpy as np

import concourse.bass as bass
import concourse.mybir as mybir
from concourse.tile import TileContext
from concourse.bass_utils import run_bass_kernel_spmd

F16 = mybir.dt.float16
F32 = mybir.dt.float32

NCORES = 8
B, C = 4, 3
BC = B * C                    # 12 bc slices
X, Y, Z = 160, 192, 160
XL = X // NCORES              # 20 x per core
CX, CY, CZ = 23, 27, 23       # control points per axis
J32 = 32                      # padded j
K32 = 32                      # padded k
NB = 4                        # partition bands
BAND_BC = 3                   # bc per band
FREE1 = BAND_BC * J32 * K32   # 3072: per-band free size of meshT/A
NGRP = BC * XL // 4           # 60 groups of 4 (bc,x) slices
NYC = Y // 32                 # 6 y-chunks
GG = 3                        # groups per MM3 supertile
NGP = NGRP // GG              # 20 supertiles
CHUNK = 512                   # MM1 free chunk (one PSUM bank of fp32)
NCH = FREE1 // CHUNK          # 6 chunks

_cache = {}


def _axis_weights3(n, sp, ncp):
    """Dense [n, ncp] matrix of 3x the cubic B-spline weights.

    With integer coordinates and spacing 8, 3*w = m/1024 with |m| <= 2048,
    exactly representable in fp16.
    """
    v = np.arange(n, dtype=np.float64) / sp
    f = np.floor(v)
    d = v - f
    w = np.stack(
        [
            (1 - d) ** 3 / 6,
            d**3 / 2 - d**2 + 2.0 / 3,
            -(d**3) / 2 + d**2 / 2 + d / 2 + 1.0 / 6,
            d**3 / 6,
        ],
        -1,
    )
    W3 = np.zeros((n, ncp))
    idx = f.astype(int)
    for a in range(4):
        W3[np.arange(n), idx + a] = np.round(3 * w[:, a] * 1024) / 1024
    assert np.all(np.float64(np.float16(W3)) == W3)
    return W3


def _host_weights():
    if "w" in _cache:
        return _cache["w"]
    W3x = _axis_weights3(X, 8, CX)
    W3y = _axis_weights3(Y, 8, CY)
    W3z = _axis_weights3(Z, 8, CZ)

    # wx3[core]: [128, 32] fp16, wxT chunk replicated on 4 partition bands
    wx3 = np.zeros((NCORES, 128, 32), np.float16)
    for core in range(NCORES):
        blk = W3x[core * XL : (core + 1) * XL, :].T.astype(np.float16)  # [23, 20]
        for q in range(NB):
            wx3[core, 32 * q : 32 * q + CX, :XL] = blk

    # wz3: [128, Z] fp16, wzT replicated on 4 partition bands
    wz3 = np.zeros((128, Z), np.float16)
    for q in range(NB):
        wz3[32 * q : 32 * q + CZ, :] = W3z.T.astype(np.float16)

    # wyb: [128, NYC*128] fp16 block-diag. Variant c covers the strided y-set
    # y = 6*yi + c (yi = 0..31): wyb[32b+j, 128c + 32b+yi] = W3y[6yi+c, j]/27.
    # The stride-6 y interleave makes each slice's (yi, c, z) staging layout
    # land contiguously in flow[y, z], so one DMA stores a whole supertile.
    # The 1/27 undoes the 3x-per-axis weight scaling here (costing ~1e-4
    # weight rounding) so the PSUM drains are pure copies with no scale.
    wyb = np.zeros((128, NYC * 128), np.float16)
    for c in range(NYC):
        for b in range(4):
            for j in range(CY):
                wyb[32 * b + j, 128 * c + 32 * b : 128 * c + 32 * b + 32] = (
                    W3y[c : c + 6 * 32 : 6, j] / 27.0
                ).astype(np.float16)

    _cache["w"] = (wx3, wz3, wyb)
    return _cache["w"]


def _prep_mesh(mesh):
    """mesh [4,3,23,27,23] f32 -> meshT [128, FREE1] fp16.

    Partition 32q+i holds mesh[bc=3q+bcq, i, j, k] at free index
    bcq*J32*K32 + j*K32 + k (j, k zero-padded to 32).
    """
    m = np.asarray(mesh, np.float32).reshape(BC, CX, CY, CZ)
    mt = np.zeros((128, BAND_BC, J32, K32), np.float32)
    for q in range(NB):
        for bcq in range(BAND_BC):
            bc = 3 * q + bcq
            mt[32 * q : 32 * q + CX, bcq, :CY, :CZ] = m[bc]
    return mt.astype(np.float16).reshape(128, FREE1)


def _build_program():
    if "nc" in _cache:
        return _cache["nc"]
    nc = bass.Bass()
    mesh16 = nc.declare_dram_parameter("mesh16", [128, FREE1], F16, isOutput=False)
    wx3 = nc.declare_dram_parameter("wx3", [128, 32], F16, isOutput=False)
    wz3 = nc.declare_dram_parameter("wz3", [128, Z], F16, isOutput=False)
    wyb = nc.declare_dram_parameter("wyb", [128, NYC * 128], F16, isOutput=False)
    flow = nc.declare_dram_parameter("flow", [BC * XL, Y, Z], F16, isOutput=True)

    # Store view: slice s = 12*gp + 4*gg + b, y = 6*yi + c. With staging laid
    # out [(b,yi) partitions, (gg,c,z) free], a whole supertile is one DMA
    # whose SBUF side is fully contiguous per partition (3x1920B dst runs).
    flowV = flow[:, :, :].rearrange(
        "(gp gg b) (yi c) z -> gp (b yi) gg c z", gg=GG, b=4, c=NYC
    )

    with TileContext(nc) as tc:
        with (
            tc.tile_pool(name="const", bufs=1) as cpool,
            tc.tile_pool(name="abuf", bufs=1) as apool,
            tc.tile_pool(name="cbuf", bufs=4) as cbpool,
            tc.tile_pool(name="stage", bufs=6) as spool,
            tc.tile_pool(name="ps12", bufs=2, space="PSUM") as ps12pool,
            tc.tile_pool(name="ps3", bufs=2, space="PSUM") as ps3pool,
        ):
            # Per-chunk mesh tiles so MM1 chunk ch starts as soon as its own
            # slice of the mesh has landed (Tile deps are whole-tile).
            wx = cpool.tile([128, 32], F16, tag="wx")
            nc.sync.dma_start(out=wx[:, :], in_=wx3[:, :])
            m2c = []
            for ch in range(NCH):
                s = slice(ch * CHUNK, (ch + 1) * CHUNK)
                t2 = cpool.tile([128, CHUNK], F16, name=f"m2{ch}", tag=f"m2{ch}")
                eng = nc.sync if ch % 2 == 0 else nc.gpsimd
                eng.dma_start(out=t2[:, :], in_=mesh16[:, s])
                m2c.append(t2)
            wz = cpool.tile([128, Z], F16, tag="wz")
            wy = cpool.tile([128, NYC * 128], F16, tag="wy")
            nc.sync.dma_start(out=wz[:, :], in_=wz3[:, :])
            nc.gpsimd.dma_start(out=wy[:, :], in_=wyb[:, :])

            # ---- MM1 (contract i) + 32x32 block transpose ----
            # Separate A tiles per bc-triple (bcq) so MM2 groups that consume
            # one triple can start while later chunks are still in MM1.
            PB = J32 * K32  # 1024: per-bcq free size
            at = [apool.tile([128, PB], F32, name=f"at{b}", tag=f"at{b}")
                  for b in range(BAND_BC)]
            # fp16 copy, stored permuted (j,x)->(x,j) so each MM2 lhsT is one
            # contiguous 128-wide run (walrus: 1 free dim).
            ah = [apool.tile([128, PB], F16, name=f"ah{b}", tag=f"ah{b}")
                  for b in range(BAND_BC)]
            for ch in range(NCH):
                p1 = ps12pool.tile([128, CHUNK], F32, tag="p12", name="p1")
                for q in range(NB):
                    band = slice(32 * q, 32 * q + CX)
                    nc.tensor.matmul(
                        p1[32 * q : 32 * q + 32, :],
                        lhsT=wx[band, :],
                        rhs=m2c[ch][band, :],
                        start=True,
                        stop=True,
                        tile_position=(32 * q, 32 * q),
                    )
                bq, half = ch // 2, (ch % 2) * CHUNK
                nc.vector.transpose(
                    out=at[bq][:, half : half + CHUNK], in_=p1[:, :]
                )
                if ch % 2 == 1:
                    b = bq
                    atP = at[b][:, :].rearrange(
                        "p (j x) -> p x j", j=J32, x=K32
                    )
                    ahV = ah[b][:, :].rearrange(
                        "p (x j) -> p x j", x=K32, j=J32
                    )
                    nc.scalar.copy(out=ahV, in_=atP)

            # ---- MM2 (contract k) + C fp16 copy + MM3 (contract j) + store --
            # Fully interleaved per gp (= one supertile of 3 slice quads):
            # MM2 produces C for this gp, MM3 consumes it immediately; the
            # fp16 staging tile is stored with one 737KB DMA per supertile.

            def emit_mm2(gp):
                """Produce C (fp16) for supertile gp."""
                p2 = ps12pool.tile([128, CHUNK], F32, tag="p12", name="p2")[:, : GG * Z]
                for sub in range(GG):
                    g = gp * GG + sub
                    bc = g // 5
                    q, bq, xg = bc // 3, bc % 3, g % 5
                    lo = 128 * xg
                    nc.tensor.matmul(
                        p2[:, sub * Z : (sub + 1) * Z],
                        lhsT=ah[bq][32 * q : 32 * q + CZ, lo : lo + 128],
                        rhs=wz[32 * q : 32 * q + CZ, :],
                        start=True,
                        stop=True,
                        tile_position=(32 * q, 0),
                    )
                chl = cbpool.tile([128, GG * Z], F16, name="chl", tag="chl")
                # PSUM drain: only scalar/vector may read PSUM.
                if gp % 3 == 0:
                    nc.scalar.copy(out=chl[:, :], in_=p2[:, :])
                else:
                    nc.vector.tensor_copy(out=chl[:, :], in_=p2[:, :])
                return chl

            def emit_mm3(gp, chl):
                """Contract j for supertile gp, scale to fp16, and store."""
                stg = spool.tile([128, GG * NYC * Z], F16, tag="stg", name="stg")
                for h in range(2):  # c-triples {0,1,2} and {3,4,5}
                    # three bank-aligned [128,480] thirds in a 3-bank tile
                    p3 = ps3pool.tile([128, 1536], F32, tag="p3", name="p3")
                    for cc in range(3):
                        c = 3 * h + cc
                        nc.tensor.matmul(
                            p3[:, cc * 512 : cc * 512 + GG * Z],
                            lhsT=wy[:, 128 * c : 128 * (c + 1)],
                            rhs=chl[:, :],
                            start=True,
                            stop=True,
                        )
                    # One fused copy drains the whole c-triple into the
                    # (gg, c, z) staging layout. Iteration (gg, cc, z): dst
                    # runs are 480 contiguous fp16 per gg; src hops PSUM
                    # banks on cc. Strict h0->DVE / h1->ACT alternation so
                    # each supertile's two drains proceed in parallel and
                    # the tensor engine never waits on a lone drain engine.
                    srcb = p3[:, :]
                    src = bass.AP(
                        srcb.tensor, srcb.offset,
                        [srcb.ap[0], [Z, GG], [512, 3], [1, Z]],
                    )
                    dstb = stg[:, 3 * h * Z : 3 * h * Z + Z]
                    dst = bass.AP(
                        dstb.tensor, dstb.offset,
                        [dstb.ap[0], [NYC * Z, GG], [Z, 3], [1, Z]],
                    )
                    if h == 0:
                        nc.vector.tensor_copy(out=dst, in_=src)
                    else:
                        nc.scalar.copy(out=dst, in_=src)
                stgV = stg[:, :].rearrange(
                    "p (gg c z) -> p gg c z", gg=GG, c=NYC
                )
                eng = nc.sync if gp % 2 == 0 else nc.gpsimd
                eng.dma_start(out=flowV[gp], in_=stgV)

            # Software pipeline (depth 2): MM3 for gp-2 is emitted after MM2
            # for gp, so C production runs well ahead of its consumption.
            pend = []
            for gp in range(NGP):
                pend.append((gp, emit_mm2(gp)))
                if len(pend) > 2:
                    emit_mm3(*pend.pop(0))
            for item in pend:
                emit_mm3(*item)

    # Walrus allows at most one sync-wait per matmul; split extras into
    # EventSemaphore instructions (same pass Bacc.compile runs).
    import bass_rust as _bass_rust

    _bass_rust.move_matmul_waits_to_ldweights(nc.m)
    _bass_rust.generate_event_semaphores(nc)

    _cache["nc"] = nc
    return nc


def _in_maps(mesh):
    wx3, wz3, wyb = _host_weights()
    mh = _prep_mesh(mesh)
    return [
        {"mesh16": mh, "wx3": wx3[core], "wz3": wz3, "wyb": wyb}
        for core in range(NCORES)
    ]


def kernel(mesh: np.ndarray) -> np.ndarray:
    nc = _build_program()
    in_maps = _in_maps(mesh)
    last_err = None
    for attempt in range(3):
        try:
            res = run_bass_kernel_spmd(nc, in_maps, list(range(NCORES))).results
            break
        except Exception as e:  # transient device wedge: retry
            last_err = e
    else:
        raise last_err
    full = np.empty((BC, X, Y, Z), np.float32)
    for core in range(NCORES):
        full[:, core * XL : (core + 1) * XL] = res[core]["flow"].reshape(
            BC, XL, Y, Z
        )
    return full.reshape(B, C, X, Y, Z)


# revision 21
# speedup vs baseline: 1.9405x; 1.1375x over previous
"""Trainium2 Bass kernel for nn_FFDGenerator: cubic B-spline free-form deformation.

Computes flow[b,c,x,y,z] = sum_{i,j,k} Wx[x,i]*Wy[y,j]*Wz[z,k]*mesh[b,c,i,j,k]
where Wx/Wy/Wz are dense per-axis cubic B-spline weight matrices (4 nonzeros
per row, spacing 8), mesh is (4,3,23,27,23), flow is (4,3,160,192,160).

Sharding: output x-axis split into 8 chunks of 20, one per NeuronCore.
Control-point mesh is replicated (bc-triples spread over partition bands).

Per-core pipeline (all contractions on the tensor engine):
  MM1: contract i -> A[xl, (bc,j,k)]       col-tiled over 4 partition bands
  T:   DVE 32x32 block transpose -> A_t[k, (bc,xl,j)] per band
  MM2: contract k -> C[(xl4,j32), (g,z)]   row-banded, data-stationary
  MM3: contract j -> out[(xl4,y32), z]     block-diag Wy weights, M=128
  Epilogue: PSUM -> SBUF fp16 copy with 1/27 scale, DMA to DRAM.

Numerics: single-pass fp16 matmuls (tolerance is 2e-2; this lands ~5e-4).
Weights are exact in fp16 via the 3x trick: 3*W has entries n/1024 with
|n| <= 2048; the 27x scale is removed in the epilogue copy. The output
crosses the wire as fp16 and is upcast to fp32 on the host, halving the
dominant HBM store traffic.
"""

import numpy as np

import concourse.bass as bass
import concourse.mybir as mybir
from concourse.tile import TileContext
from concourse.bass_utils import run_bass_kernel_spmd

F16 = mybir.dt.float16
F32 = mybir.dt.float32

NCORES = 8
B, C = 4, 3
BC = B * C                    # 12 bc slices
X, Y, Z = 160, 192, 160
XL = X // NCORES              # 20 x per core
CX, CY, CZ = 23, 27, 23       # control points per axis
J32 = 32                      # padded j
K32 = 32                      # padded k
NB = 4                        # partition bands
BAND_BC = 3                   # bc per band
FREE1 = BAND_BC * J32 * K32   # 3072: per-band free size of meshT/A
NGRP = BC * XL // 4           # 60 groups of 4 (bc,x) slices
NYC = Y // 32                 # 6 y-chunks
GG = 3                        # groups per MM3 supertile
NGP = NGRP // GG              # 20 supertiles
CHUNK = 512                   # MM1 free chunk (one PSUM bank of fp32)
NCH = FREE1 // CHUNK          # 6 chunks

_cache = {}


def _axis_weights3(n, sp, ncp):
    """Dense [n, ncp] matrix of 3x the cubic B-spline weights.

    With integer coordinates and spacing 8, 3*w = m/1024 with |m| <= 2048,
    exactly representable in fp16.
    """
    v = np.arange(n, dtype=np.float64) / sp
    f = np.floor(v)
    d = v - f
    w = np.stack(
        [
            (1 - d) ** 3 / 6,
            d**3 / 2 - d**2 + 2.0 / 3,
            -(d**3) / 2 + d**2 / 2 + d / 2 + 1.0 / 6,
            d**3 / 6,
        ],
        -1,
    )
    W3 = np.zeros((n, ncp))
    idx = f.astype(int)
    for a in range(4):
        W3[np.arange(n), idx + a] = np.round(3 * w[:, a] * 1024) / 1024
    assert np.all(np.float64(np.float16(W3)) == W3)
    return W3


def _host_weights():
    if "w" in _cache:
        return _cache["w"]
    W3x = _axis_weights3(X, 8, CX)
    W3y = _axis_weights3(Y, 8, CY)
    W3z = _axis_weights3(Z, 8, CZ)

    # wx3[core]: [128, 32] fp16, wxT chunk replicated on 4 partition bands
    wx3 = np.zeros((NCORES, 128, 32), np.float16)
    for core in range(NCORES):
        blk = W3x[core * XL : (core + 1) * XL, :].T.astype(np.float16)  # [23, 20]
        for q in range(NB):
            wx3[core, 32 * q : 32 * q + CX, :XL] = blk

    # wz3: [128, Z] fp16, wzT replicated on 4 partition bands
    wz3 = np.zeros((128, Z), np.float16)
    for q in range(NB):
        wz3[32 * q : 32 * q + CZ, :] = W3z.T.astype(np.float16)

    # wyb: [128, NYC*128] fp16 block-diag. Variant c covers the strided y-set
    # y = 6*yi + c (yi = 0..31): wyb[32b+j, 128c + 32b+yi] = W3y[6yi+c, j]/27.
    # The stride-6 y interleave makes each slice's (yi, c, z) staging layout
    # land contiguously in flow[y, z], so one DMA stores a whole supertile.
    # The 1/27 undoes the 3x-per-axis weight scaling here (costing ~1e-4
    # weight rounding) so the PSUM drains are pure copies with no scale.
    wyb = np.zeros((128, NYC * 128), np.float16)
    for c in range(NYC):
        for b in range(4):
            for j in range(CY):
                wyb[32 * b + j, 128 * c + 32 * b : 128 * c + 32 * b + 32] = (
                    W3y[c : c + 6 * 32 : 6, j] / 27.0
                ).astype(np.float16)

    _cache["w"] = (wx3, wz3, wyb)
    return _cache["w"]


def _prep_mesh(mesh):
    """mesh [4,3,23,27,23] f32 -> meshT [128, FREE1] fp16.

    Partition 32q+i holds mesh[bc=3q+bcq, i, j, k] at free index
    bcq*J32*K32 + j*K32 + k (j, k zero-padded to 32).
    """
    m = np.asarray(mesh, np.float32).reshape(BC, CX, CY, CZ)
    mt = np.zeros((128, BAND_BC, J32, K32), np.float32)
    for q in range(NB):
        for bcq in range(BAND_BC):
            bc = 3 * q + bcq
            mt[32 * q : 32 * q + CX, bcq, :CY, :CZ] = m[bc]
    return mt.astype(np.float16).reshape(128, FREE1)


def _build_program():
    if "nc" in _cache:
        return _cache["nc"]
    nc = bass.Bass()
    mesh16 = nc.declare_dram_parameter("mesh16", [128, FREE1], F16, isOutput=False)
    wx3 = nc.declare_dram_parameter("wx3", [128, 32], F16, isOutput=False)
    wz3 = nc.declare_dram_parameter("wz3", [128, Z], F16, isOutput=False)
    wyb = nc.declare_dram_parameter("wyb", [128, NYC * 128], F16, isOutput=False)
    flow = nc.declare_dram_parameter("flow", [BC * XL, Y, Z], F16, isOutput=True)

    # Store view: slice s = 12*gp + 4*gg + b, y = 6*yi + c. With staging laid
    # out [(b,yi) partitions, (gg,c,z) free], a whole supertile is one DMA
    # whose SBUF side is fully contiguous per partition (3x1920B dst runs).
    flowV = flow[:, :, :].rearrange(
        "(gp gg b) (yi c) z -> gp (b yi) gg c z", gg=GG, b=4, c=NYC
    )

    with TileContext(nc) as tc:
        with (
            tc.tile_pool(name="const", bufs=1) as cpool,
            tc.tile_pool(name="abuf", bufs=1) as apool,
            tc.tile_pool(name="cbuf", bufs=4) as cbpool,
            tc.tile_pool(name="stage", bufs=6) as spool,
            tc.tile_pool(name="ps12", bufs=2, space="PSUM") as ps12pool,
            tc.tile_pool(name="ps3", bufs=3, space="PSUM") as ps3pool,
        ):
            # Per-chunk mesh tiles so MM1 chunk ch starts as soon as its own
            # slice of the mesh has landed (Tile deps are whole-tile). Load
            # triggers are spread across engines: a dma_start occupies its
            # issuing queue ~0.6us, so serializing them on sync delays MM1.
            wx = cpool.tile([128, 32], F16, tag="wx")
            nc.scalar.dma_start(out=wx[:, :], in_=wx3[:, :])
            m2c = []
            load_engs = [nc.sync, nc.gpsimd, nc.scalar, nc.sync, nc.gpsimd,
                         nc.scalar]
            for ch in range(NCH):
                s = slice(ch * CHUNK, (ch + 1) * CHUNK)
                t2 = cpool.tile([128, CHUNK], F16, name=f"m2{ch}", tag=f"m2{ch}")
                load_engs[ch].dma_start(out=t2[:, :], in_=mesh16[:, s])
                m2c.append(t2)
            wz = cpool.tile([128, Z], F16, tag="wz")
            wy = cpool.tile([128, NYC * 128], F16, tag="wy")
            nc.gpsimd.dma_start(out=wz[:, :], in_=wz3[:, :])
            nc.sync.dma_start(out=wy[:, :], in_=wyb[:, :])

            # ---- MM1 (contract i) + 32x32 block transpose ----
            # Separate A tiles per bc-triple (bcq) so MM2 groups that consume
            # one triple can start while later chunks are still in MM1.
            PB = J32 * K32  # 1024: per-bcq free size
            at = [apool.tile([128, PB], F32, name=f"at{b}", tag=f"at{b}")
                  for b in range(BAND_BC)]
            # fp16 copy, stored permuted (j,x)->(x,j) so each MM2 lhsT is one
            # contiguous 128-wide run (walrus: 1 free dim).
            ah = [apool.tile([128, PB], F16, name=f"ah{b}", tag=f"ah{b}")
                  for b in range(BAND_BC)]
            for ch in range(NCH):
                p1 = ps12pool.tile([128, CHUNK], F32, tag="p12", name="p1")
                for q in range(NB):
                    band = slice(32 * q, 32 * q + CX)
                    nc.tensor.matmul(
                        p1[32 * q : 32 * q + 32, :],
                        lhsT=wx[band, :],
                        rhs=m2c[ch][band, :],
                        start=True,
                        stop=True,
                        tile_position=(32 * q, 32 * q),
                    )
                bq, half = ch // 2, (ch % 2) * CHUNK
                nc.vector.transpose(
                    out=at[bq][:, half : half + CHUNK], in_=p1[:, :]
                )
                if ch % 2 == 1:
                    b = bq
                    atP = at[b][:, :].rearrange(
                        "p (j x) -> p x j", j=J32, x=K32
                    )
                    ahV = ah[b][:, :].rearrange(
                        "p (x j) -> p x j", x=K32, j=J32
                    )
                    nc.scalar.copy(out=ahV, in_=atP)

            # ---- MM2 (contract k) + C fp16 copy + MM3 (contract j) + store --
            # Fully interleaved per gp (= one supertile of 3 slice quads):
            # MM2 produces C for this gp, MM3 consumes it immediately; the
            # fp16 staging tile is stored with one 737KB DMA per supertile.

            def emit_mm2(gp):
                """Produce C (fp16) for supertile gp."""
                p2 = ps12pool.tile([128, CHUNK], F32, tag="p12", name="p2")[:, : GG * Z]
                for sub in range(GG):
                    g = gp * GG + sub
                    bc = g // 5
                    q, bq, xg = bc // 3, bc % 3, g % 5
                    lo = 128 * xg
                    nc.tensor.matmul(
                        p2[:, sub * Z : (sub + 1) * Z],
                        lhsT=ah[bq][32 * q : 32 * q + CZ, lo : lo + 128],
                        rhs=wz[32 * q : 32 * q + CZ, :],
                        start=True,
                        stop=True,
                        tile_position=(32 * q, 0),
                    )
                chl = cbpool.tile([128, GG * Z], F16, name="chl", tag="chl")
                # PSUM drain on ACT: DVE already carries two of the three
                # MM3 pair-drains per supertile.
                nc.scalar.copy(out=chl[:, :], in_=p2[:, :])
                return chl

            def emit_mm3(gp, chl):
                """Contract j for supertile gp, scale to fp16, and store."""
                stg = spool.tile([128, GG * NYC * Z], F16, tag="stg", name="stg")
                for pr in range(3):  # c-pairs {0,1}, {2,3}, {4,5}
                    # two bank-aligned [128,480] halves in a 2-bank tile;
                    # bufs=3 rotation gives each drain ~2 pair-phases of
                    # slack before the tensor engine needs the banks back.
                    p3 = ps3pool.tile([128, 1024], F32, tag="p3", name="p3")
                    for cc in range(2):
                        c = 2 * pr + cc
                        nc.tensor.matmul(
                            p3[:, cc * 512 : cc * 512 + GG * Z],
                            lhsT=wy[:, 128 * c : 128 * (c + 1)],
                            rhs=chl[:, :],
                            start=True,
                            stop=True,
                        )
                    # One fused copy drains the c-pair into the (gg, c, z)
                    # staging layout. Iteration (gg, cc, z): dst runs are
                    # 320 contiguous fp16 per gg; src hops PSUM banks on cc.
                    # DVE takes two pairs per gp, ACT one: matches their
                    # drain rates (~1.23 vs ~1.0 rows/ns).
                    srcb = p3[:, :]
                    src = bass.AP(
                        srcb.tensor, srcb.offset,
                        [srcb.ap[0], [Z, GG], [512, 2], [1, Z]],
                    )
                    dstb = stg[:, 2 * pr * Z : 2 * pr * Z + Z]
                    dst = bass.AP(
                        dstb.tensor, dstb.offset,
                        [dstb.ap[0], [NYC * Z, GG], [Z, 2], [1, Z]],
                    )
                    if pr == 1:
                        nc.scalar.copy(out=dst, in_=src)
                    else:
                        nc.vector.tensor_copy(out=dst, in_=src)
                stgV = stg[:, :].rearrange(
                    "p (gg c z) -> p gg c z", gg=GG, c=NYC
                )
                eng = nc.sync if gp % 2 == 0 else nc.gpsimd
                eng.dma_start(out=flowV[gp], in_=stgV)

            # Software pipeline (depth 2): MM3 for gp-2 is emitted after MM2
            # for gp, so C production runs well ahead of its consumption.
            pend = []
            for gp in range(NGP):
                pend.append((gp, emit_mm2(gp)))
                if len(pend) > 2:
                    emit_mm3(*pend.pop(0))
            for item in pend:
                emit_mm3(*item)

    # Walrus allows at most one sync-wait per matmul; split extras into
    # EventSemaphore instructions (same pass Bacc.compile runs).
    import bass_rust as _bass_rust

    _bass_rust.move_matmul_waits_to_ldweights(nc.m)
    _bass_rust.generate_event_semaphores(nc)

    _cache["nc"] = nc
    return nc


def _in_maps(mesh):
    wx3, wz3, wyb = _host_weights()
    mh = _prep_mesh(mesh)
    return [
        {"mesh16": mh, "wx3": wx3[core], "wz3": wz3, "wyb": wyb}
        for core in range(NCORES)
    ]


def kernel(mesh: np.ndarray) -> np.ndarray:
    nc = _build_program()
    in_maps = _in_maps(mesh)
    last_err = None
    for attempt in range(3):
        try:
            res = run_bass_kernel_spmd(nc, in_maps, list(range(NCORES))).results
            break
        except Exception as e:  # transient device wedge: retry
            last_err = e
    else:
        raise last_err
    full = np.empty((BC, X, Y, Z), np.float32)
    for core in range(NCORES):
        full[:, core * XL : (core + 1) * XL] = res[core]["flow"].reshape(
            BC, XL, Y, Z
        )
    return full.reshape(B, C, X, Y, Z)
